# revision 26
# baseline (speedup 1.0000x reference)
"""LocallyConnectedXYZLayer Trainium2 kernel.

out[n,c,i,j] = sum_{dh,dw in 5x5} sm[n,c,i+dh,(j+dw)%W] * mask[...] *
               exp(-||xyz[:,i+dh,(j+dw)%W] - xyz[:,i,j]||^2 / 2)
(zero-padded in H, circular in W)

Factorization used on device:
  exp(-d2/2) = exp(cross) * phi_src * phi_ctr,  phi = exp(-|xyz|^2/2),
  cross = x_s*x_c + y_s*y_c + z_s*z_c
so   out = phi_ctr * sum_k  psi_s[c] * exp(cross_k),
     psi[c] = sm[c] * mask * phi       (all per-pixel maps)

Sharding: 8 cores, each takes the full N=2 x H=64 rows (interleaved on the
128 SBUF partitions as p = i*2 + n so dh row-shifts are partition shifts
that never cross batches) and a 256-column W chunk with +-2 halo (circular).

The run is dominated by the axon tunnel (~25-55 MB/s per direction), so
I/O is minimized: xyz ships as fp16 and softmax as 6-bit (the {0,1} mask
and the round(sm*63) quantization are pre-folded on the host; 4 channels
pack into 3 bytes, grouped along C so masked pixels stay zero-byte runs
for the wire compression), all in a single u8 input tensor; the output
ships as uint8 with a dynamic per-partition scale packed into the same
tensor (4 f32 bytes per row).
The donated zero output buffers of the stock run_bass_kernel_spmd path
are dropped (the kernel writes every output element), the jitted
executable is cached across calls, and the per-shard D2H copies are
kicked off async so dequant/unshard overlaps the remaining transfers.

The 25-offset channel MAC runs on the vector engine with fp16 psi, f32
exp(cross), and an f32 accumulator (psi stored twice at even alignment so
every dw window read stays 4B-aligned for 16-bit mode); device exec is a
negligible share of the call, so precision is free.
"""

import sys

sys.path.insert(0, "/opt/trn_rl_repo")

import numpy as np

N, C, H, W = 2, 20, 64, 2048
NCORES = 8
WC = W // NCORES          # 256 columns per core
WH = WC + 4               # with halo
P = H * N                 # 128 partitions
FS = C * WC               # 5120 output values per row
FS6 = FS * 3 // 4         # 3840 bytes after 6-bit packing
NQ = FS // 4              # 1280 value-quads per row
OSCALE = 62.99            # quant scale: acc*s + 0.5 stays < 63.5
CHUNKS = ((0, 2), (2, 4), (4, 6), (6, 8))   # pipelined core groups

_CACHE = {}


def _build():
    import concourse.bass as bass
    import concourse.mybir as mybir
    from concourse.tile import TileContext
    from concourse import tile as tile_mod
    from concourse.vector_clock import ScopedClock

    # --- walrus in this env rejects >2 sem-waits on one CTRL inst: put the
    # final-drain waits on a chain of nops (2 waits each) instead.
    def _patched_dab(self, tick_clock, wait_clock):
        nc = self.nc
        carrier = nc.sync.nop(nofuse=True, hint="drain_waits")
        wait_clock.add_sem_waits(
            carrier.ins, ScopedClock({None: tick_clock.global_clock})
        )
        si = carrier.ins.sync_info
        if si is not None and len(si.on_wait) > 2:
            waits = list(si.on_wait)
            carrier.ins.sync_info = mybir.SyncInfo(
                on_wait=waits[:2], on_update=list(si.on_update)
            )
            rest = waits[2:]
            while rest:
                chunk, rest = rest[:2], rest[2:]
                extra = nc.sync.nop(nofuse=True, hint="drain_waits")
                extra.ins.sync_info = mybir.SyncInfo(on_wait=chunk, on_update=[])
        nc.sync.drain()
        nc.all_engine_barrier()
        popped = nc._tile_sem_poison_stack.pop()
        assert popped is self._sem_poison
        nc.clear_and_free_semaphores(list(self.sems.allocated().values()))
        nc.all_engine_barrier()

    tile_mod.TileContext._drain_and_barrier = _patched_dab

    def split_excess_waits(nc, max_waits=1):
        for f in nc.m.functions:
            for blk in f.blocks:
                insts = blk.instructions
                i = 0
                while i < len(insts):
                    inst = insts[i]
                    si = inst.sync_info
                    if si is not None and len(si.on_wait) > max_waits:
                        waits = list(si.on_wait)
                        keep = waits[:max_waits]
                        extra = waits[max_waits:]
                        k = 0
                        while extra:
                            chunk = extra[:max_waits]
                            extra = extra[max_waits:]
                            nop = mybir.InstNoOp(
                                name=f"{inst.name}_ws{k}",
                                engine=inst.engine, ins=[], outs=[],
                                sync_info=mybir.SyncInfo(on_wait=chunk,
                                                         on_update=[]),
                            )
                            insts.insert(i, nop)
                            i += 1
                            k += 1
                        inst.sync_info = mybir.SyncInfo(
                            on_wait=keep, on_update=list(si.on_update))
                    i += 1

    f32 = mybir.dt.float32
    f16 = mybir.dt.float16
    u8 = mybir.dt.uint8
    mult = mybir.AluOpType.mult
    add = mybir.AluOpType.add
    mx = mybir.AluOpType.max
    Exp = mybir.ActivationFunctionType.Exp
    Square = mybir.ActivationFunctionType.Square
    Copy = mybir.ActivationFunctionType.Copy

    nc = bass.Bass("TRN2", target_bir_lowering=False, debug=False,
                   num_devices=NCORES)
    AND = mybir.AluOpType.bitwise_and
    OR = mybir.AluOpType.bitwise_or
    SHL = mybir.AluOpType.logical_shift_left
    SHR = mybir.AluOpType.logical_shift_right

    # one packed input / one packed output to minimize axon round trips:
    # cin = [xyz 10-bit/coord, one u32 per pixel | sm 5-bit/channel,
    # 13 bytes per pixel], oout = [6-bit codes | scale f32 bytes].
    # Pixel-major records keep masked pixels as 13-byte zero runs that
    # the H2D wire compression can eat.
    XB = 4 * WH                         # 1040 bytes of packed coords
    SB = 13 * WH                        # 3380 packed softmax bytes
    cin = nc.declare_dram_parameter("cin", [P, XB + SB], u8, isOutput=False)
    oout = nc.declare_dram_parameter("oout", [P, FS6 + 4], u8, isOutput=True)

    def view(t, poff, pc, off, dims):
        a = t[:]
        pstride = a.ap[0][0]
        return bass.AP(a.tensor, a.offset + poff * pstride + off,
                       [[pstride, pc]] + dims)

    with TileContext(nc) as tc:
        with tc.tile_pool(name="main", bufs=1) as pool, \
             tc.tile_pool(name="cross", bufs=2) as cpool, \
             tc.tile_pool(name="tmps", bufs=2) as tpool, \
             tc.tile_pool(name="shift", bufs=1) as spool:
            xt_b = pool.tile([P, XB], u8)
            nc.sync.dma_start(out=xt_b[:], in_=cin[:, 0:XB])
            smp = pool.tile([P, SB], u8)
            nc.sync.dma_start(out=smp[:], in_=cin[:, XB:XB + SB])
            # unpack 13-byte pixel records -> 20 channels of 5-bit sm codes;
            # channel c sits at bit 5c of the little-endian 104-bit record
            smt_q = pool.tile([P, C * WH], u8)
            tub = pool.tile([P, WH], u8)
            for c in range(C):
                bit = 5 * c
                j0, s = bit >> 3, bit & 7
                vout = view(smt_q, 0, P, c * WH, [[1, WH]])
                blo = view(smp, 0, P, j0, [[13, WH]])
                if s == 0:
                    nc.vector.tensor_scalar(vout, blo, 31, None, AND)
                elif s <= 3:
                    nc.vector.tensor_scalar(vout, blo, s, 31, SHR, AND)
                else:
                    bhi = view(smp, 0, P, j0 + 1, [[13, WH]])
                    nc.vector.tensor_scalar(tub[:], blo, s, None, SHR)
                    mask_bits = (1 << (s + 5 - 8)) - 1
                    nc.vector.tensor_scalar(vout, bhi, mask_bits, 8 - s,
                                            AND, SHL)
                    nc.vector.tensor_tensor(vout, vout, tub[:], OR)
            # u8 -> fp16 (values 0..31 exact; the /31 dequant and the host
            # quant scale are both folded into the host-side final divide)
            smt_h = pool.tile([P, C * WH], f16)
            nc.scalar.copy(smt_h[:], smt_q[:])

            # 10-bit coords: u32 pixel word = x | y<<10 | z<<20, dequant
            # v = code * (12/1024) - 6
            xt = pool.tile([P, 3 * WH], f32)
            bx = [view(xt_b, 0, P, t, [[4, WH]]) for t in range(4)]
            XSTEP, XR = 12.0 / 1024.0, 6.0
            ua = pool.tile([P, WH], u8)
            fa = pool.tile([P, WH], f32)
            fb = pool.tile([P, WH], f32)
            for d, (lo_src, lo_op, hi_src, hi_op, hi_mul) in enumerate((
                    (0, None, 1, ("and", 3), 256.0),      # x: b0 | (b1&3)<<8
                    (1, ("shr", 2), 2, ("and", 15), 64.0),  # y
                    (2, ("shr", 4), 3, None, 16.0))):     # z
                if lo_op is None:
                    nc.scalar.copy(fa[:], bx[lo_src])
                else:
                    nc.vector.tensor_scalar(ua[:], bx[lo_src], lo_op[1],
                                            None, SHR)
                    nc.scalar.copy(fa[:], ua[:])
                if hi_op is None:
                    nc.scalar.copy(fb[:], bx[hi_src])
                else:
                    nc.vector.tensor_scalar(ua[:], bx[hi_src], hi_op[1],
                                            None, AND)
                    nc.scalar.copy(fb[:], ua[:])
                nc.vector.tensor_scalar(fb[:], fb[:], hi_mul, None, mult)
                nc.vector.tensor_tensor(fa[:], fa[:], fb[:], add)
                nc.scalar.activation(xt[:, d * WH:(d + 1) * WH], fa[:],
                                     Copy, scale=XSTEP, bias=-XR)

            # q = x^2+y^2+z^2 -> phi = exp(-q/2)
            sq0 = pool.tile([P, WH], f32)
            sq1 = pool.tile([P, WH], f32)
            nc.scalar.activation(sq0[:], xt[:, 0:WH], Square)
            nc.scalar.activation(sq1[:], xt[:, WH:2 * WH], Square)
            nc.vector.tensor_add(sq0[:], sq0[:], sq1[:])
            nc.scalar.activation(sq1[:], xt[:, 2 * WH:3 * WH], Square)
            nc.vector.tensor_add(sq0[:], sq0[:], sq1[:])
            phi = pool.tile([P, WH], f32)
            nc.scalar.activation(phi[:], sq0[:], Exp, scale=-0.5)

            # psi[c] = sm255[c] * phi (mask pre-folded into sm on host; the
            # x255 scale rides through to the dynamic output scale), stored
            # twice in fp16: psiA at column parity 0, psiB pre-shifted by one
            # column, so dw in {0,2,4} reads psiA and dw in {1,3} reads psiB
            # at even element offsets (4B-aligned for DVE 2x mode).
            psiA = pool.tile([P, C * WH], f16)
            psiB = pool.tile([P, C * WH], f16)
            phi_bc = view(phi, 0, P, 0, [[0, C], [1, WH]])
            smt_v = view(smt_h, 0, P, 0, [[WH, C], [1, WH]])
            nc.vector.tensor_tensor(
                view(psiA, 0, P, 0, [[WH, C], [1, WH]]), smt_v, phi_bc, mult)
            # psiB[., c, j] = psiA[., c, j+1]; DMA has no alignment limits
            nc.sync.dma_start(
                out=view(psiB, 0, P, 0, [[WH, C], [1, WH - 1]]),
                in_=view(psiA, 0, P, 1, [[WH, C], [1, WH - 1]]))

            accV = pool.tile([P, FS], f32)    # f32 accumulator chain

            for dh in (0, -1, 1, -2, 2):
                pc = P - 2 * abs(dh)
                pi = max(0, 2 * dh)    # source partition offset
                po = max(0, -2 * dh)   # dest partition offset
                if dh == 0:
                    pA, pB, xs_t = psiA, psiB, xt
                else:
                    # row-shifted copies via DMA (engines cannot start an AP
                    # at partition % 32 != 0); memset first so the out-of-
                    # range rows read as zero.
                    pA = spool.tile([P, C * WH], f16, tag="pA")
                    pB = spool.tile([P, C * WH], f16, tag="pB")
                    xs_t = spool.tile([P, 3 * WH], f32, tag="xs")
                    nc.vector.memset(pA[:], 0.0)
                    nc.vector.memset(pB[:], 0.0)
                    nc.vector.memset(xs_t[:], 0.0)
                    nc.sync.dma_start(out=pA[po:po + pc, :],
                                      in_=psiA[pi:pi + pc, :])
                    nc.sync.dma_start(out=pB[po:po + pc, :],
                                      in_=psiB[pi:pi + pc, :])
                    nc.sync.dma_start(out=xs_t[po:po + pc, :],
                                      in_=xt[pi:pi + pc, :])
                # cross terms for all 5 dw at once: [P, 5, 256] f32
                m1 = cpool.tile([P, 5 * WC], f32, tag="m1")
                m2 = cpool.tile([P, 5 * WC], f32, tag="m2")
                m3 = cpool.tile([P, 5 * WC], f32, tag="m3")
                for d, mm in enumerate((m1, m2, m3)):
                    xs = view(xs_t, 0, P, d * WH, [[1, 5], [1, WC]])
                    xc = view(xt, 0, P, d * WH + 2, [[0, 5], [1, WC]])
                    mo = view(mm, 0, P, 0, [[WC, 5], [1, WC]])
                    nc.vector.tensor_tensor(mo, xs, xc, mult)
                v1 = view(m1, 0, P, 0, [[WC, 5], [1, WC]])
                v2 = view(m2, 0, P, 0, [[WC, 5], [1, WC]])
                v3 = view(m3, 0, P, 0, [[WC, 5], [1, WC]])
                nc.vector.tensor_tensor(v1, v1, v2, add)
                nc.vector.tensor_tensor(v1, v1, v3, add)
                ee = cpool.tile([P, 5 * WC], f32, tag="ee")
                ev = view(ee, 0, P, 0, [[WC, 5], [1, WC]])
                nc.scalar.activation(ev, v1, Exp)

                for dw in range(5):
                    src_t = pA if dw % 2 == 0 else pB
                    soff = dw if dw % 2 == 0 else dw - 1
                    ps = view(src_t, 0, P, soff, [[WH, C], [1, WC]])
                    eb = view(ee, 0, P, dw * WC, [[0, C], [1, WC]])
                    av = view(accV, 0, P, 0, [[WC, C], [1, WC]])
                    if dh == 0 and dw == 0:
                        nc.vector.tensor_tensor(av, ps, eb, mult)
                        continue
                    tmp = tpool.tile([P, FS], f32, tag="tmp")
                    tv = view(tmp, 0, P, 0, [[WC, C], [1, WC]])
                    nc.vector.tensor_tensor(tv, ps, eb, mult)
                    nc.vector.tensor_tensor(av, av, tv, add)

            # scale by phi_center in place, then quantize to u8 with a
            # per-partition dynamic scale (packed into the output bytes).
            ov = view(accV, 0, P, 0, [[WC, C], [1, WC]])
            pb = view(phi, 0, P, 2, [[0, C], [1, WC]])
            nc.vector.tensor_tensor(ov, ov, pb, mult)
            tmax = pool.tile([P, 1], f32)
            nc.vector.tensor_reduce(tmax[:], accV[:], mybir.AxisListType.X,
                                    mx)
            nc.vector.tensor_scalar_max(tmax[:], tmax[:], 1e-30)
            nc.sync.dma_start(out=oout[:, FS6:FS6 + 4],
                              in_=tmax[:].bitcast(u8))
            trec = pool.tile([P, 1], f32)
            nc.vector.reciprocal(trec[:], tmax[:])
            tsc = pool.tile([P, 1], f32)
            # OSCALE so v*s + 0.5 can never reach 63.5 (6-bit codes)
            nc.vector.tensor_scalar_mul(tsc[:], trec[:], OSCALE)
            out_q = pool.tile([P, FS], u8)
            nc.scalar.activation(out_q[:], accV[:], Copy, bias=0.0,
                                 scale=tsc[:])
            # pack 4 consecutive 6-bit codes -> 3 bytes (little-endian 24b)
            out_p = pool.tile([P, FS6], u8)
            qv = [view(out_q, 0, P, m, [[4, NQ]]) for m in range(4)]
            bv = [view(out_p, 0, P, t, [[3, NQ]]) for t in range(3)]
            tA = pool.tile([P, NQ], u8)
            tB = pool.tile([P, NQ], u8)
            nc.vector.tensor_scalar(tA[:], qv[1], 6, None, SHL)  # u8 wrap
            nc.vector.tensor_tensor(bv[0], qv[0], tA[:], OR)
            nc.vector.tensor_scalar(tA[:], qv[1], 2, None, SHR)
            nc.vector.tensor_scalar(tB[:], qv[2], 4, None, SHL)
            nc.vector.tensor_tensor(bv[1], tA[:], tB[:], OR)
            nc.vector.tensor_scalar(tA[:], qv[2], 4, None, SHR)
            nc.vector.tensor_scalar(tB[:], qv[3], 2, None, SHL)
            nc.vector.tensor_tensor(bv[2], tA[:], tB[:], OR)
            nc.sync.dma_start(out=oout[:, 0:FS6], in_=out_p[:])

    split_excess_waits(nc)
    return nc


def _get_runner():
    """Build nc + the jitted SPMD executor once; cache for warm calls."""
    if "runner" in _CACHE:
        return _CACHE["runner"]
    import jax
    from jax.sharding import Mesh, PartitionSpec
    from jax.experimental.shard_map import shard_map
    from concourse import bass2jax
    import concourse.mybir as mybir

    nc = _build()
    bass2jax.install_neuronx_cc_hook()
    partition_name = (nc.partition_id_tensor.name
                      if nc.partition_id_tensor else None)
    in_names, out_names, out_avals = [], [], []
    for alloc in nc.m.functions[0].allocations:
        if not isinstance(alloc, mybir.MemoryLocationSet):
            continue
        name = alloc.memorylocations[0].name
        if alloc.kind == "ExternalInput":
            if name != partition_name:
                in_names.append(name)
        elif alloc.kind == "ExternalOutput":
            out_names.append(name)
            out_avals.append(jax.core.ShapedArray(
                tuple(alloc.tensor_shape), mybir.dt.np(alloc.dtype)))
    bind_names = tuple(in_names) + ((partition_name,) if partition_name
                                    else ())

    def _body(*args):
        operands = list(args)
        if partition_name is not None:
            operands.append(bass2jax.partition_id_tensor())
        outs = bass2jax._bass_exec_p.bind(
            *operands,
            out_avals=tuple(out_avals),
            in_names=bind_names,
            out_names=tuple(out_names),
            lowering_input_output_aliases=(),
            sim_require_finite=True,
            sim_require_nnan=True,
            nc=nc,
        )
        return tuple(outs)

    devices = jax.devices()[:NCORES]
    fns = []
    for a, b in CHUNKS:
        mesh = Mesh(np.asarray(devices[a:b]), ("core",))
        fns.append(jax.jit(shard_map(
            _body, mesh=mesh,
            in_specs=(PartitionSpec("core"),) * len(in_names),
            out_specs=(PartitionSpec("core"),) * len(out_names),
            check_rep=False)))
    _CACHE["runner"] = (fns, in_names, out_names)
    return _CACHE["runner"]


XB = 4 * WH                     # bytes of 10-bit packed coords per row
SB = 13 * WH                    # 5-bit packed softmax bytes per row
ROWB = XB + SB                  # bytes per cin row (4420)
XSTEP, XR = 12.0 / 1024.0, 6.0  # coord dequant: v = code*XSTEP - XR

# C fast path for the quantize+mask+pack (and output unpack) inner loops;
# compiled on first use, with the numpy implementation as fallback.
_CSRC = r"""
#include <string.h>
#include <stdint.h>

/* cin rows (core k, partition p=h*2+n) of 4420 bytes:
   [260 pixels * 4B of 10-bit coords | 260 pixels * 13B of 5-bit sm] */

/* sm: (2,20,64,2048) f32, mask: (2,64,2048) i32; channel c at bit 5c
   of the little-endian 104-bit pixel record */
void pack_sm(const float* sm, const int32_t* mask, unsigned char* cin,
             int k0, int k1) {
    for (int k = k0; k < k1; ++k)
      for (int h = 0; h < 64; ++h)
        for (int n = 0; n < 2; ++n) {
          unsigned char* row =
              cin + (size_t)((k * 64 + h) * 2 + n) * 4420 + 1040;
          const int32_t* mrow = mask + ((size_t)n * 64 + h) * 2048;
          const float* srow = sm + ((size_t)n * 20 * 64 + h) * 2048;
          for (int j = 0; j < 260; ++j) {
            int col = (k * 256 + j - 2 + 2048) & 2047;
            unsigned char* px = row + (size_t)j * 13;
            if (!mrow[col]) { memset(px, 0, 13); continue; }
            uint64_t w0 = 0, w1 = 0;
            for (int c = 0; c < 12; ++c)
              w0 |= (uint64_t)(unsigned)(srow[(size_t)c * 131072 + col]
                                         * 31.0f + 0.5f) << (5 * c);
            for (int c = 12; c < 20; ++c)
              w1 |= (uint64_t)(unsigned)(srow[(size_t)c * 131072 + col]
                                         * 31.0f + 0.5f) << (5 * (c - 12));
            for (int t = 0; t < 7; ++t) px[t] = (w0 >> (8 * t)) & 255;
            px[7] = ((w0 >> 56) & 15) | ((w1 & 15) << 4);
            uint64_t r = w1 >> 4;
            for (int t = 0; t < 5; ++t) px[8 + t] = (r >> (8 * t)) & 255;
          }
        }
}

/* xyz: (2,3,64,2048) f32 -> u32 pixel word x | y<<10 | z<<20,
   code = clip(round((v+6)*1024/12), 0, 1023) */
void pack_xyz(const float* xyz, unsigned char* cin, int k0, int k1) {
    for (int k = k0; k < k1; ++k)
      for (int h = 0; h < 64; ++h)
        for (int n = 0; n < 2; ++n) {
          unsigned char* row =
              cin + (size_t)((k * 64 + h) * 2 + n) * 4420;
          const float* xr = xyz + ((size_t)n * 3 * 64 + h) * 2048;
          for (int j = 0; j < 260; ++j) {
            int col = (k * 256 + j - 2 + 2048) & 2047;
            uint32_t w = 0;
            for (int d = 0; d < 3; ++d) {
              float t = (xr[(size_t)d * 131072 + col] + 6.0f)
                        * (1024.0f / 12.0f) + 0.5f;
              int q = (int)t;
              if (q < 0) q = 0;
              if (q > 1023) q = 1023;
              w |= (uint32_t)q << (10 * d);
            }
            unsigned char* px = row + (size_t)j * 4;
            px[0] = w & 255; px[1] = (w >> 8) & 255;
            px[2] = (w >> 16) & 255; px[3] = (w >> 24) & 255;
          }
        }
}

/* qk: (128, 3844) u8 shard (6-bit packed + 4B scale), sc: (128,) f32
   dequant scale, out: (2,20,64,2048) f32, writes cols [k*256,(k+1)*256) */
void unpack_out(const unsigned char* qk, const float* sc, float* out,
                int k) {
    for (int p = 0; p < 128; ++p) {
      int h = p >> 1, n = p & 1;
      const unsigned char* q = qk + (size_t)p * 3844;
      float s = sc[p];
      for (int c = 0; c < 20; ++c) {
        float* o = out + (((size_t)n * 20 + c) * 64 + h) * 2048 + k * 256;
        const unsigned char* qq = q + (size_t)c * 192;
        for (int t = 0; t < 64; ++t) {
          unsigned b0 = qq[3 * t], b1 = qq[3 * t + 1], b2 = qq[3 * t + 2];
          o[4 * t]     = (float)(b0 & 63) * s;
          o[4 * t + 1] = (float)((b0 >> 6) | ((b1 & 15) << 2)) * s;
          o[4 * t + 2] = (float)((b1 >> 4) | ((b2 & 3) << 4)) * s;
          o[4 * t + 3] = (float)(b2 >> 2) * s;
        }
      }
    }
}
"""


def _get_clib():
    if "clib" in _CACHE:
        return _CACHE["clib"]
    lib = None
    try:
        import subprocess, tempfile, ctypes, os
        d = tempfile.mkdtemp(prefix="lcxyz_")
        src = os.path.join(d, "pack.c")
        so = os.path.join(d, "pack.so")
        with open(src, "w") as f:
            f.write(_CSRC)
        subprocess.run(["cc", "-O3", "-shared", "-fPIC", src, "-o", so],
                       check=True, capture_output=True, timeout=120)
        lib = ctypes.CDLL(so)
        lib.pack_sm.argtypes = [ctypes.c_void_p] * 3 + [ctypes.c_int] * 2
        lib.pack_sm.restype = None
        lib.pack_xyz.argtypes = [ctypes.c_void_p] * 2 + [ctypes.c_int] * 2
        lib.pack_xyz.restype = None
        lib.unpack_out.argtypes = [ctypes.c_void_p] * 3 + [ctypes.c_int]
        lib.unpack_out.restype = None
    except Exception:
        lib = None
    _CACHE["clib"] = lib
    return lib


def _get_prep_bufs():
    b = _CACHE.get("prep_bufs")
    if b is None:
        from numpy.lib.stride_tricks import as_strided
        cin = np.empty((NCORES * P, ROWB), np.uint8)
        b = _CACHE["prep_bufs"] = {
            "cin": cin,
            "f32": np.empty((N, C, H, W + 4), np.float32),
            "s_e": np.empty((N, C, H, W + 4), np.uint8),
            "q_e": np.empty((N, 3, H, W + 4), np.uint16),
            # byte-plane views aliasing the packed buffer (numpy fallback)
            "xvp": np.ndarray((NCORES, H, N, WH, 4), np.uint8,
                              buffer=cin.data, offset=0,
                              strides=(H * N * ROWB, N * ROWB, ROWB,
                                       4, 1)),
            "svp": np.ndarray((NCORES, H, N, WH, 13), np.uint8,
                              buffer=cin.data, offset=XB,
                              strides=(H * N * ROWB, N * ROWB, ROWB,
                                       13, 1)),
        }

        def win_view(a_e):  # (N, CD, H, W+4) -> (8, H, N, CD, WH) view
            t = a_e.transpose(2, 0, 1, 3)
            st = t.strides
            return as_strided(t, shape=(NCORES, H, N, a_e.shape[1], WH),
                              strides=(WC * st[3], st[0], st[1], st[2],
                                       st[3]))

        b["sviews"] = win_view(b["s_e"])
        b["qviews"] = win_view(b["q_e"])
    return b


def _prep_chunk(b, a_core, b_core, xyz, sm, mk, mk32):
    """Quantize/pack only the W-slice needed by cores [a_core, b_core).

    Straight serial numpy/C: this container has a single CPU core, so
    thread pools only add churn.
    """
    lib = _get_clib()
    if lib is not None:
        lib.pack_xyz(xyz.ctypes.data, b["cin"].ctypes.data, a_core, b_core)
        lib.pack_sm(sm.ctypes.data, mk32.ctypes.data,
                    b["cin"].ctypes.data, a_core, b_core)
        return
    # ---- numpy fallback: quantize into halo-extended buffers, then pack
    c0 = a_core * WC
    c1 = min(b_core * WC + 2, W)         # +2: right halo of the last core
    s_e, q_e, buf = b["s_e"], b["q_e"], b["f32"]

    def qx(dst, src):
        # +0.5 then truncate == C's round-half-up
        np.copyto(dst, np.clip((src + XR) * (1.0 / XSTEP) + 0.5,
                               0, 1023), casting="unsafe")

    def qs(dst_u8, src, msl):
        bs = buf[..., :src.shape[-1]]
        np.multiply(src, 31.0, out=bs)
        bs += 0.5
        np.copyto(dst_u8, bs, casting="unsafe")   # f32->u8 trunc = round
        dst_u8 *= msl

    if a_core == 0:
        # left wrap halo: ext[...,0:2] = core cols W-2..W
        qx(q_e[..., 0:2], xyz[..., W - 2:W])
        qs(s_e[..., 0:2], sm[..., W - 2:W], mk[..., W - 2:W])
    qx(q_e[..., c0 + 2:c1 + 2], xyz[..., c0:c1])
    qs(s_e[..., c0 + 2:c1 + 2], sm[..., c0:c1], mk[..., c0:c1])
    if b_core == NCORES:
        # right wrap halo: ext cols W+2..W+4 = core cols 0..2 (from chunk 0)
        q_e[..., W + 2:] = q_e[..., 2:4]
        s_e[..., W + 2:] = s_e[..., 2:4]
    svp, sviews = b["svp"], b["sviews"]
    xvp, qviews = b["xvp"], b["qviews"]
    for k in range(a_core, b_core):
        qv = qviews[k]                           # (H, N, 3, WH) u16
        w = (qv[:, :, 0, :].astype(np.uint32)
             | (qv[:, :, 1, :].astype(np.uint32) << 10)
             | (qv[:, :, 2, :].astype(np.uint32) << 20))
        for t in range(4):
            xvp[k, ..., t] = (w >> (8 * t)).astype(np.uint8)
        sv = sviews[k].astype(np.uint64)         # (H, N, C, WH)
        w0 = sv[:, :, 0, :].copy()
        for c in range(1, 12):
            w0 |= sv[:, :, c, :] << (5 * c)
        w1 = sv[:, :, 12, :].copy()
        for c in range(13, 20):
            w1 |= sv[:, :, c, :] << (5 * (c - 12))
        for t in range(7):
            svp[k, ..., t] = (w0 >> (8 * t)).astype(np.uint8)
        svp[k, ..., 7] = (((w0 >> 56) & 15) | ((w1 & 15) << 4)).astype(
            np.uint8)
        r = w1 >> 4
        for t in range(5):
            svp[k, ..., 8 + t] = (r >> (8 * t)).astype(np.uint8)


def kernel(xyz, softmax, mask):
    fns, in_names, out_names = _get_runner()
    oi = out_names.index("oout")
    xyz = np.asarray(xyz, np.float32)
    sm = np.ascontiguousarray(np.asarray(softmax, np.float32))
    mk32 = np.ascontiguousarray(np.asarray(mask, np.int32))
    lib = _get_clib()
    mk = (None if lib is not None
          else mk32.astype(np.uint8)[:, None])                 # (N,1,H,W)
    b = _get_prep_bufs()
    cin = b["cin"]
    # pipelined: prep + dispatch chunk k, then prep k+1 while k uploads;
    # fetch/dequant chunk k while later chunks are still in flight
    chunk_outs = []
    for ci, (a, bb) in enumerate(CHUNKS):
        _prep_chunk(b, a, bb, xyz, sm, mk, mk32)
        arrs = fns[ci](cin[a * P:bb * P])
        pk = arrs[oi]
        for s in pk.addressable_shards:
            s.data.copy_to_host_async()
        chunk_outs.append(pk)
    out = np.empty((N, C, H, W), np.float32)
    ub = _CACHE.get("unpack_buf")
    if ub is None:
        ub = _CACHE["unpack_buf"] = np.empty((P, NQ, 4), np.uint8)
    for ci, (a, bb) in enumerate(CHUNKS):
        shards = sorted(chunk_outs[ci].addressable_shards,
                        key=lambda s: s.index[0].start or 0)
        for kk, s in enumerate(shards):
            k = a + kk
            qk = np.asarray(s.data)                            # (P, FS6+4) u8
            mx = qk[:, FS6:].copy().view(np.float32)           # (P, 1)
            # dequant: device acc = 31*out_true, q ~= acc * OSCALE/max
            scv = mx * (1.0 / (OSCALE * 31.0))                 # (P, 1) f32
            if lib is not None:
                qkc = qk if qk.flags.c_contiguous else np.ascontiguousarray(qk)
                lib.unpack_out(qkc.ctypes.data, scv.ctypes.data,
                               out.ctypes.data, k)
                continue
            # unpack 3 bytes -> 4 six-bit codes (numpy fallback)
            q3 = qk[:, :FS6].reshape(P, NQ, 3)
            b0, b1, b2 = q3[..., 0], q3[..., 1], q3[..., 2]
            ub[..., 0] = b0 & 63
            ub[..., 1] = (b0 >> 6) | ((b1 & 15) << 2)
            ub[..., 2] = (b1 >> 4) | ((b2 & 3) << 4)
            ub[..., 3] = b2 >> 2
            sc = scv.reshape(H, N)                             # per (i, n)
            np.multiply(
                ub.reshape(P, FS).reshape(H, N, C, WC).transpose(1, 2, 0, 3),
                sc.transpose(1, 0)[:, None, :, None],
                out=out[:, :, :, k * WC:(k + 1) * WC],
                dtype=np.float32)
    return out

